# revision 4
# baseline (speedup 1.0000x reference)
"""AdaptiveTopKSelector Trainium2 kernel (8 NeuronCores, SPMD data-parallel).

Computes, for scores [4, 2048, 8192]:
  indices [4, 2048, 4096] int32 : per-row top-4096 indices, descending score,
                                  ties broken by lower index (jax.lax.top_k)
  mask    [4, 2048, 4096] bool  : prefix mask pos < k_adaptive(row)

Strategy: shard the 8192 (batch*seq_q) rows across 8 cores (1024 rows each,
8 tiles of [128 rows x 8192]).  Per tile, a normalized descending bitonic
sort (91 stages) runs on the Vector engine over (value fp32, index uint16)
pairs, with the two unconditional index base-copies offloaded to the Scalar
engine.  Equal-value runs are then re-ordered by index with 4 odd-even
fixup passes.  The adaptive k needs a global mean of per-row variances, so
phase A reduces per-row variance per shard and a single-scalar AllReduce
combines shard sums across the 8 cores.
"""

import os
import sys

for _p in ("/opt/trn_rl_repo", "/root/.axon_site/_ro/trn_rl_repo"):
    if os.path.isdir(_p) and _p not in sys.path:
        sys.path.append(_p)

import numpy as np

N = 8192          # row length (seq_kv)
K_EFF = 4096      # output k
ROWS = 8192       # total rows (4 * 2048)
NCORES = 8
ROWS_PER_CORE = ROWS // NCORES          # 1024
TILES = ROWS_PER_CORE // 128            # 8
FIXUP_PASSES = 4

_CACHE = {}


def _build():
    import concourse.bacc as bacc
    import concourse.mybir as mybir
    from concourse.tile import TileContext
    from bass_rust import ReduceOp as _ReduceOp

    f32 = mybir.dt.float32
    u16 = mybir.dt.uint16
    u8 = mybir.dt.uint8
    i32 = mybir.dt.int32
    Alu = mybir.AluOpType

    nc = bacc.Bacc("TRN2", target_bir_lowering=False, debug=False,
                   num_devices=NCORES)

    scores = nc.dram_tensor("scores", [ROWS_PER_CORE, N], f32,
                            kind="ExternalInput")
    idx_out = nc.dram_tensor("idx", [ROWS_PER_CORE, K_EFF], i32,
                             kind="ExternalOutput")
    mask_out = nc.dram_tensor("mask", [ROWS_PER_CORE, K_EFF], u8,
                              kind="ExternalOutput")
    vs_in = nc.dram_tensor("vs_in", [1, 1], f32)
    vs_out = nc.dram_tensor("vs_out", [1, 1], f32, addr_space="Shared")

    sc = scores.ap().rearrange("(t p) n -> t p n", p=128)
    io = idx_out.ap().rearrange("(t p) n -> t p n", p=128)
    mo = mask_out.ap().rearrange("(t p) n -> t p n", p=128)

    with TileContext(nc) as tc:
        with tc.tile_pool(name="big", bufs=1) as big:
            val0 = big.tile([128, N], f32, tag="val0")
            val1 = big.tile([128, N], f32, tag="val1")
            idx0 = big.tile([128, N], u16, tag="idx0")
            idx1 = big.tile([128, N], u16, tag="idx1")
            iota16 = big.tile([128, N], u16, tag="iota16")
            pos32 = big.tile([128, K_EFF], f32, tag="pos32")
            cbuf = big.tile([128, N // 2], u8, tag="cbuf")
            c2buf = big.tile([128, N // 2], u8, tag="c2buf")
            ftmp = big.tile([128, N // 2], u16, tag="ftmp")
            iostage = big.tile([128, K_EFF], i32, tag="iostage")
            mbuf = big.tile([128, K_EFF], u8, tag="mbuf")
            stats = big.tile([128, 32], f32, tag="stats")
            # stats columns: 0:8 sumsq, 8:16 mean, 16:24 km1, 24.. scratch

            sumsq = stats[:, 0:8]
            means = stats[:, 8:16]
            km1 = stats[:, 16:24]
            scr = stats[:, 24:32]

            nc.gpsimd.iota(iota16[:], pattern=[[1, N]], base=0,
                           channel_multiplier=0,
                           allow_small_or_imprecise_dtypes=True)
            nc.gpsimd.iota(pos32[:], pattern=[[1, K_EFF]], base=0,
                           channel_multiplier=0,
                           allow_small_or_imprecise_dtypes=True)

            # ---------- phase A: per-row variance ----------
            for t in range(TILES):
                buf = val0 if t % 2 == 0 else val1
                tmp = val1 if t % 2 == 0 else val0
                nc.sync.dma_start(out=buf[:], in_=sc[t])
                nc.vector.reduce_sum(scr[:, 0:1], buf[:],
                                     axis=mybir.AxisListType.X)
                nc.vector.tensor_scalar(means[:, t:t + 1], scr[:, 0:1],
                                        1.0 / N, None, op0=Alu.mult)
                nc.vector.tensor_scalar(tmp[:], buf[:], means[:, t:t + 1],
                                        None, op0=Alu.subtract)
                nc.vector.tensor_mul(tmp[:], tmp[:], tmp[:])
                nc.vector.reduce_sum(sumsq[:, t:t + 1], tmp[:],
                                     axis=mybir.AxisListType.X)

            # var = sumsq/(N-1); shard var sum -> AllReduce -> global mean
            nc.vector.tensor_scalar(sumsq[:], sumsq[:], 1.0 / (N - 1), None,
                                    op0=Alu.mult)
            nc.vector.reduce_sum(scr[:, 1:2], sumsq[:],
                                 axis=mybir.AxisListType.X)
            nc.gpsimd.partition_all_reduce(scr[:, 2:3], scr[:, 1:2],
                                           channels=128,
                                           reduce_op=_ReduceOp.add)
            nc.sync.dma_start(out=vs_in[:], in_=scr[0:1, 2:3])
            nc.gpsimd.collective_compute(
                "AllReduce", Alu.add,
                replica_groups=[list(range(NCORES))],
                ins=[vs_in.ap()], outs=[vs_out.ap()])
            nc.sync.dma_start(out=scr[0:1, 3:4], in_=vs_out[:])
            nc.gpsimd.partition_broadcast(scr[:, 4:5], scr[0:1, 3:4])
            # inv = 1/(gsum/ROWS + 1e-8)
            nc.vector.tensor_scalar(scr[:, 5:6], scr[:, 4:5], 1.0 / ROWS,
                                    1e-8, op0=Alu.mult, op1=Alu.add)
            nc.vector.reciprocal(scr[:, 6:7], scr[:, 5:6])
            # km1 = clip(2048*(0.5 + 1/(1 + var*inv)), 256, 4096) - 1
            nc.vector.tensor_scalar(km1[:], sumsq[:], scr[:, 6:7], 1.0,
                                    op0=Alu.mult, op1=Alu.add)
            nc.vector.reciprocal(km1[:], km1[:])
            nc.vector.tensor_scalar(km1[:], km1[:], 0.5, 2048.0,
                                    op0=Alu.add, op1=Alu.mult)
            nc.vector.tensor_scalar(km1[:], km1[:], 4096.0, 256.0,
                                    op0=Alu.min, op1=Alu.max)
            nc.vector.tensor_scalar(km1[:], km1[:], 1.0, None,
                                    op0=Alu.subtract)

            # ---------- phase B: sort + outputs ----------
            def stage(va, ia, vb, ib, nva, nia, nvb, nib, L):
                # compare+exchange: max -> A position, min -> B position.
                # c = (va >= vb); value min/max on DVE; index base copies on
                # ACT (crossed), then predicated copies (straight) on DVE.
                cv = cbuf[:, 0:L]
                cap = cv.rearrange("p (a b) -> p a b", b=va.shape[-1]) \
                    if len(va.shape) == 3 else cv
                nc.vector.tensor_tensor(cap, va, vb, op=Alu.is_ge)
                nc.vector.tensor_tensor(nvb, va, vb, op=Alu.min)
                nc.vector.tensor_tensor(nva, va, vb, op=Alu.max)
                nc.scalar.copy(nia, ib)
                nc.scalar.copy(nib, ia)
                nc.vector.copy_predicated(nia, cap, ia)
                nc.vector.copy_predicated(nib, cap, ib)

            for t in range(TILES):
                vbufs = [val0, val1]
                ibufs = [idx0, idx1]
                nc.sync.dma_start(out=val0[:], in_=sc[t])
                nc.vector.tensor_copy(idx0[:], iota16[:])
                s = 0  # current buffer index
                for lev in range(13):
                    K = 1 << lev
                    vc, vn = vbufs[s], vbufs[1 - s]
                    ic, inx = ibufs[s], ibufs[1 - s]
                    # mirror stage over 2K blocks
                    g = vc[:].rearrange("p (b k) -> p b k", k=2 * K)
                    gi = ic[:].rearrange("p (b k) -> p b k", k=2 * K)
                    go = vn[:].rearrange("p (b k) -> p b k", k=2 * K)
                    goi = inx[:].rearrange("p (b k) -> p b k", k=2 * K)
                    def rev_half(x):
                        # reversed second half of each 2K block: 2K-1 .. K
                        return x[:, :, 2 * K - 1:K - 1:-1]
                    va, vb = g[:, :, 0:K], rev_half(g)
                    ia, ib = gi[:, :, 0:K], rev_half(gi)
                    nva, nvb = go[:, :, 0:K], rev_half(go)
                    nia, nib = goi[:, :, 0:K], rev_half(goi)
                    stage(va, ia, vb, ib, nva, nia, nvb, nib, N // 2)
                    s = 1 - s
                    # substages
                    j = K // 2
                    while j >= 1:
                        vc, vn = vbufs[s], vbufs[1 - s]
                        ic, inx = ibufs[s], ibufs[1 - s]
                        g = vc[:].rearrange("p (b k) -> p b k", k=2 * j)
                        gi = ic[:].rearrange("p (b k) -> p b k", k=2 * j)
                        go = vn[:].rearrange("p (b k) -> p b k", k=2 * j)
                        goi = inx[:].rearrange("p (b k) -> p b k", k=2 * j)
                        stage(g[:, :, 0:j], gi[:, :, 0:j],
                              g[:, :, j:2 * j], gi[:, :, j:2 * j],
                              go[:, :, 0:j], goi[:, :, 0:j],
                              go[:, :, j:2 * j], goi[:, :, j:2 * j],
                              N // 2)
                        s = 1 - s
                        j //= 2

                vf, iff = vbufs[s], ibufs[s]
                # ---------- tie fixup: order equal-value runs by index ----
                for p in range(FIXUP_PASSES):
                    par = p % 2
                    L = (N - par) // 2 * 2
                    npair = L // 2
                    va = vf[:, par:par + L].rearrange("p (a b) -> p a b", b=2)
                    ia = iff[:, par:par + L].rearrange("p (a b) -> p a b", b=2)
                    A_v, B_v = va[:, :, 0:1], va[:, :, 1:2]
                    A_i, B_i = ia[:, :, 0:1], ia[:, :, 1:2]
                    ceq = cbuf[:, 0:npair].rearrange("p (a b) -> p a b", b=1)
                    cgt = c2buf[:, 0:npair].rearrange("p (a b) -> p a b", b=1)
                    ft = ftmp[:, 0:npair].rearrange("p (a b) -> p a b", b=1)
                    nc.vector.tensor_tensor(ceq, A_v, B_v, op=Alu.is_equal)
                    nc.vector.tensor_tensor(cgt, A_i, B_i, op=Alu.is_gt)
                    nc.vector.tensor_tensor(ceq, ceq, cgt, op=Alu.mult)
                    nc.vector.tensor_copy(ft, A_i)
                    nc.vector.copy_predicated(A_i, ceq, B_i)
                    nc.vector.copy_predicated(B_i, ceq, ft)

                # ---------- outputs ----------
                nc.vector.tensor_copy(iostage[:], iff[:, 0:K_EFF])
                nc.sync.dma_start(out=io[t], in_=iostage[:])
                nc.vector.tensor_scalar(mbuf[:], pos32[:], km1[:, t:t + 1],
                                        None, op0=Alu.is_le)
                nc.sync.dma_start(out=mo[t], in_=mbuf[:])

    nc.compile()
    return nc


def _get_nc():
    if "nc" not in _CACHE:
        _CACHE["nc"] = _build()
    return _CACHE["nc"]


def kernel(scores, seq_q=None, seq_kv=None, **_ignored):
    from concourse.bass_utils import run_bass_kernel_spmd

    scores = np.ascontiguousarray(np.asarray(scores), dtype=np.float32)
    B, Q, Nk = scores.shape
    assert (B * Q, Nk) == (ROWS, N), f"unexpected shape {scores.shape}"
    rows = scores.reshape(ROWS, N)

    nc = _get_nc()
    in_maps = [
        {"scores": rows[c * ROWS_PER_CORE:(c + 1) * ROWS_PER_CORE]}
        for c in range(NCORES)
    ]
    res = run_bass_kernel_spmd(nc, in_maps, core_ids=list(range(NCORES)))
    idx = np.concatenate([res.results[c]["idx"] for c in range(NCORES)], axis=0)
    mask = np.concatenate([res.results[c]["mask"] for c in range(NCORES)], axis=0)
    idx = idx.reshape(B, Q, K_EFF).astype(np.int32, copy=False)
    mask = mask.reshape(B, Q, K_EFF).astype(bool)
    return idx, mask


if __name__ == "__main__":
    s = np.load("/tmp/scores.npy")
    i, m = kernel(s, 2048, 8192)
    print(i.shape, i.dtype, m.shape, m.dtype)


# revision 8
# speedup vs baseline: 1.0803x; 1.0803x over previous
"""AdaptiveTopKSelector Trainium2 kernel (8 NeuronCores, SPMD data-parallel).

Computes, for scores [4, 2048, 8192]:
  indices [4, 2048, 4096] int32 : per-row top-4096 indices, descending score,
                                  ties broken by lower index (jax.lax.top_k)
  mask    [4, 2048, 4096] bool  : prefix mask pos < k_adaptive(row)

Strategy: shard the 8192 (batch*seq_q) rows across 8 cores (1024 rows each,
8 tiles of [128 rows x 8192]).  Per tile, a normalized descending bitonic
sort (91 stages) runs on the Vector engine over (value fp32, index uint16)
pairs, with the two unconditional index base-copies offloaded to the Scalar
engine.  Equal-value runs are then re-ordered by index with 4 odd-even
fixup passes.  The adaptive k needs a global mean of per-row variances, so
phase A reduces per-row variance per shard and a single-scalar AllReduce
combines shard sums across the 8 cores.
"""

import os
import sys

for _p in ("/opt/trn_rl_repo", "/root/.axon_site/_ro/trn_rl_repo"):
    if os.path.isdir(_p) and _p not in sys.path:
        sys.path.append(_p)

import numpy as np

N = 8192          # row length (seq_kv)
K_EFF = 4096      # output k
ROWS = 8192       # total rows (4 * 2048)
NCORES = 8
ROWS_PER_CORE = ROWS // NCORES          # 1024
TILES = ROWS_PER_CORE // 128            # 8
FIXUP_PASSES = 4

_CACHE = {}


def _build():
    import concourse.bacc as bacc
    import concourse.mybir as mybir
    from concourse.tile import TileContext
    from bass_rust import ReduceOp as _ReduceOp

    f32 = mybir.dt.float32
    u16 = mybir.dt.uint16
    u8 = mybir.dt.uint8
    i32 = mybir.dt.int32
    Alu = mybir.AluOpType

    nc = bacc.Bacc("TRN2", target_bir_lowering=False, debug=False,
                   num_devices=NCORES)

    scores = nc.dram_tensor("scores", [ROWS_PER_CORE, N], f32,
                            kind="ExternalInput")
    idx_out = nc.dram_tensor("idx", [ROWS_PER_CORE, K_EFF], i32,
                             kind="ExternalOutput")
    mask_out = nc.dram_tensor("mask", [ROWS_PER_CORE, K_EFF], u8,
                              kind="ExternalOutput")
    vs_in = nc.dram_tensor("vs_in", [1, 1], f32)
    vs_out = nc.dram_tensor("vs_out", [1, 1], f32, addr_space="Shared")

    sc = scores.ap().rearrange("(t p) n -> t p n", p=128)
    io = idx_out.ap().rearrange("(t p) n -> t p n", p=128)
    mo = mask_out.ap().rearrange("(t p) n -> t p n", p=128)

    with TileContext(nc) as tc:
        with tc.tile_pool(name="big", bufs=1) as big:
            val0 = big.tile([128, N], f32, tag="val0")
            val1 = big.tile([128, N], f32, tag="val1")
            idx0 = big.tile([128, N], u16, tag="idx0")
            idx1 = big.tile([128, N], u16, tag="idx1")
            iota16 = big.tile([128, N], u16, tag="iota16")
            pos32 = big.tile([128, K_EFF], f32, tag="pos32")
            cbuf = big.tile([128, N // 2], u8, tag="cbuf")
            c2buf = big.tile([128, N // 2], u8, tag="c2buf")
            ftmp = big.tile([128, N // 2], u16, tag="ftmp")
            iostage = big.tile([128, K_EFF], i32, tag="iostage")
            mbuf = big.tile([128, K_EFF], u8, tag="mbuf")
            stats = big.tile([128, 32], f32, tag="stats")
            stats16 = big.tile([128, 4], u16, tag="stats16")
            # stats columns: 0:8 sumsq, 8:16 mean, 16:24 km1, 24.. scratch

            sumsq = stats[:, 0:8]
            means = stats[:, 8:16]
            km1 = stats[:, 16:24]
            scr = stats[:, 24:32]

            nc.gpsimd.iota(iota16[:], pattern=[[1, N]], base=0,
                           channel_multiplier=0,
                           allow_small_or_imprecise_dtypes=True)
            nc.gpsimd.iota(pos32[:], pattern=[[1, K_EFF]], base=0,
                           channel_multiplier=0,
                           allow_small_or_imprecise_dtypes=True)

            # ---------- phase A: per-row variance ----------
            for t in range(TILES):
                buf = val0 if t % 2 == 0 else val1
                tmp = val1 if t % 2 == 0 else val0
                nc.sync.dma_start(out=buf[:], in_=sc[t])
                nc.vector.reduce_sum(scr[:, 0:1], buf[:],
                                     axis=mybir.AxisListType.X)
                nc.vector.tensor_scalar(means[:, t:t + 1], scr[:, 0:1],
                                        1.0 / N, None, op0=Alu.mult)
                nc.vector.tensor_scalar(tmp[:], buf[:], means[:, t:t + 1],
                                        None, op0=Alu.subtract)
                nc.vector.tensor_mul(tmp[:], tmp[:], tmp[:])
                nc.vector.reduce_sum(sumsq[:, t:t + 1], tmp[:],
                                     axis=mybir.AxisListType.X)

            # var = sumsq/(N-1); shard var sum -> AllReduce -> global mean
            nc.vector.tensor_scalar(sumsq[:], sumsq[:], 1.0 / (N - 1), None,
                                    op0=Alu.mult)
            nc.vector.reduce_sum(scr[:, 1:2], sumsq[:],
                                 axis=mybir.AxisListType.X)
            nc.gpsimd.partition_all_reduce(scr[:, 2:3], scr[:, 1:2],
                                           channels=128,
                                           reduce_op=_ReduceOp.add)
            nc.sync.dma_start(out=vs_in[:], in_=scr[0:1, 2:3])
            nc.gpsimd.collective_compute(
                "AllReduce", Alu.add,
                replica_groups=[list(range(NCORES))],
                ins=[vs_in.ap()], outs=[vs_out.ap()])
            nc.sync.dma_start(out=scr[0:1, 3:4], in_=vs_out[:])
            nc.gpsimd.partition_broadcast(scr[:, 4:5], scr[0:1, 3:4])
            # inv = 1/(gsum/ROWS + 1e-8)
            nc.vector.tensor_scalar(scr[:, 5:6], scr[:, 4:5], 1.0 / ROWS,
                                    1e-8, op0=Alu.mult, op1=Alu.add)
            nc.vector.reciprocal(scr[:, 6:7], scr[:, 5:6])
            # km1 = clip(2048*(0.5 + 1/(1 + var*inv)), 256, 4096) - 1
            nc.vector.tensor_scalar(km1[:], sumsq[:], scr[:, 6:7], 1.0,
                                    op0=Alu.mult, op1=Alu.add)
            nc.vector.reciprocal(km1[:], km1[:])
            nc.vector.tensor_scalar(km1[:], km1[:], 0.5, 2048.0,
                                    op0=Alu.add, op1=Alu.mult)
            nc.vector.tensor_scalar(km1[:], km1[:], 4096.0, 256.0,
                                    op0=Alu.min, op1=Alu.max)
            nc.vector.tensor_scalar(km1[:], km1[:], 1.0, None,
                                    op0=Alu.subtract)

            # ---------- phase B: sort + outputs ----------
            def stage(va, ia, vb, ib, nva, nia, nvb, nib, L):
                # compare+exchange: max -> A position, min -> B position.
                # c = (va >= vb); value min/max on DVE; index base copies on
                # ACT (crossed), then predicated copies (straight) on DVE.
                cv = cbuf[:, 0:L]
                cap = cv.rearrange("p (a b) -> p a b", b=va.shape[-1]) \
                    if len(va.shape) == 3 else cv
                nc.vector.tensor_tensor(cap, va, vb, op=Alu.is_ge)
                nc.vector.tensor_tensor(nvb, va, vb, op=Alu.min)
                nc.vector.tensor_tensor(nva, va, vb, op=Alu.max)
                nc.scalar.copy(nia, ib)
                nc.scalar.copy(nib, ia)
                nc.vector.copy_predicated(nia, cap, ia)
                nc.vector.copy_predicated(nib, cap, ib)

            for t in range(TILES):
                vbufs = [val0, val1]
                ibufs = [idx0, idx1]
                nc.sync.dma_start(out=val0[:], in_=sc[t])
                nc.vector.tensor_copy(idx0[:], iota16[:])
                s = 0  # current buffer index
                for lev in range(13):
                    K = 1 << lev
                    vc, vn = vbufs[s], vbufs[1 - s]
                    ic, inx = ibufs[s], ibufs[1 - s]
                    # mirror stage over 2K blocks
                    g = vc[:].rearrange("p (b k) -> p b k", k=2 * K)
                    gi = ic[:].rearrange("p (b k) -> p b k", k=2 * K)
                    go = vn[:].rearrange("p (b k) -> p b k", k=2 * K)
                    goi = inx[:].rearrange("p (b k) -> p b k", k=2 * K)
                    def rev_half(x):
                        # reversed second half of each 2K block: 2K-1 .. K
                        return x[:, :, 2 * K - 1:K - 1:-1]
                    va, vb = g[:, :, 0:K], rev_half(g)
                    ia, ib = gi[:, :, 0:K], rev_half(gi)
                    nva, nvb = go[:, :, 0:K], rev_half(go)
                    nia, nib = goi[:, :, 0:K], rev_half(goi)
                    stage(va, ia, vb, ib, nva, nia, nvb, nib, N // 2)
                    s = 1 - s
                    # substages; on the final level only the top half (the
                    # output K_EFF) needs to finish sorting -- the bottom half
                    # stays at its post-mirror state in the final buffer.
                    W = K_EFF if K == N // 2 else N
                    j = K // 2
                    while j >= 1:
                        vc, vn = vbufs[s], vbufs[1 - s]
                        ic, inx = ibufs[s], ibufs[1 - s]
                        g = vc[:, 0:W].rearrange("p (b k) -> p b k", k=2 * j)
                        gi = ic[:, 0:W].rearrange("p (b k) -> p b k", k=2 * j)
                        go = vn[:, 0:W].rearrange("p (b k) -> p b k", k=2 * j)
                        goi = inx[:, 0:W].rearrange("p (b k) -> p b k", k=2 * j)
                        stage(g[:, :, 0:j], gi[:, :, 0:j],
                              g[:, :, j:2 * j], gi[:, :, j:2 * j],
                              go[:, :, 0:j], goi[:, :, 0:j],
                              go[:, :, j:2 * j], goi[:, :, j:2 * j],
                              W // 2)
                        s = 1 - s
                        j //= 2

                vf, iff = vbufs[s], ibufs[s]
                # ---------- boundary surgery ------------------------------
                # The bottom half was left unsorted (post-mirror bitonic).
                # If its max equals the value at position K_EFF-1 (a tie run
                # straddling the cut), the reference keeps the lowest-indexed
                # members: swap in the bottom's lowest-index tied element if
                # it beats the boundary element's index.  (The subsequent
                # fixup passes then order the run by index.)
                bs = stats[:, 28:32]  # scratch (phase-A scr cols are dead now)
                nc.vector.reduce_sum(bs[:, 0:1], vf[:, K_EFF:N],
                                     axis=mybir.AxisListType.X, op=Alu.max)
                nc.vector.tensor_scalar(cbuf[:, 0:N // 2], vf[:, K_EFF:N],
                                        bs[:, 0:1], None, op0=Alu.is_equal)
                nc.vector.memset(ftmp[:, 0:N // 2], 16383)
                nc.vector.copy_predicated(ftmp[:, 0:N // 2],
                                          cbuf[:, 0:N // 2], iff[:, K_EFF:N])
                nc.vector.reduce_sum(stats16[:, 0:1], ftmp[:, 0:N // 2],
                                     axis=mybir.AxisListType.X, op=Alu.min)
                # cond = (bottom_max == v[K_EFF-1]) & (rmin < idx[K_EFF-1])
                nc.vector.tensor_tensor(cbuf[:, 0:1], bs[:, 0:1],
                                        vf[:, K_EFF - 1:K_EFF], op=Alu.is_equal)
                nc.vector.tensor_tensor(c2buf[:, 0:1], stats16[:, 0:1],
                                        iff[:, K_EFF - 1:K_EFF], op=Alu.is_lt)
                nc.vector.tensor_tensor(cbuf[:, 0:1], cbuf[:, 0:1],
                                        c2buf[:, 0:1], op=Alu.mult)
                nc.vector.copy_predicated(iff[:, K_EFF - 1:K_EFF],
                                          cbuf[:, 0:1], stats16[:, 0:1])
                # ---------- tie fixup: order equal-value runs by index ----
                for p in range(FIXUP_PASSES):
                    par = p % 2
                    L = (K_EFF - par) // 2 * 2
                    npair = L // 2
                    va = vf[:, par:par + L].rearrange("p (a b) -> p a b", b=2)
                    ia = iff[:, par:par + L].rearrange("p (a b) -> p a b", b=2)
                    A_v, B_v = va[:, :, 0:1], va[:, :, 1:2]
                    A_i, B_i = ia[:, :, 0:1], ia[:, :, 1:2]
                    ceq = cbuf[:, 0:npair].rearrange("p (a b) -> p a b", b=1)
                    cgt = c2buf[:, 0:npair].rearrange("p (a b) -> p a b", b=1)
                    ft = ftmp[:, 0:npair].rearrange("p (a b) -> p a b", b=1)
                    nc.vector.tensor_tensor(ceq, A_v, B_v, op=Alu.is_equal)
                    nc.vector.tensor_tensor(cgt, A_i, B_i, op=Alu.is_gt)
                    nc.vector.tensor_tensor(ceq, ceq, cgt, op=Alu.mult)
                    nc.vector.tensor_copy(ft, A_i)
                    nc.vector.copy_predicated(A_i, ceq, B_i)
                    nc.vector.copy_predicated(B_i, ceq, ft)

                # ---------- outputs ----------
                nc.vector.tensor_copy(iostage[:], iff[:, 0:K_EFF])
                nc.sync.dma_start(out=io[t], in_=iostage[:])
                nc.vector.tensor_scalar(mbuf[:], pos32[:], km1[:, t:t + 1],
                                        None, op0=Alu.is_le)
                nc.sync.dma_start(out=mo[t], in_=mbuf[:])

    nc.compile()
    return nc


def _get_nc():
    if "nc" not in _CACHE:
        _CACHE["nc"] = _build()
    return _CACHE["nc"]


def kernel(scores, seq_q=None, seq_kv=None, **_ignored):
    from concourse.bass_utils import run_bass_kernel_spmd

    scores = np.ascontiguousarray(np.asarray(scores), dtype=np.float32)
    B, Q, Nk = scores.shape
    assert (B * Q, Nk) == (ROWS, N), f"unexpected shape {scores.shape}"
    rows = scores.reshape(ROWS, N)

    nc = _get_nc()
    in_maps = [
        {"scores": rows[c * ROWS_PER_CORE:(c + 1) * ROWS_PER_CORE]}
        for c in range(NCORES)
    ]
    res = run_bass_kernel_spmd(nc, in_maps, core_ids=list(range(NCORES)))
    idx = np.concatenate([res.results[c]["idx"] for c in range(NCORES)], axis=0)
    mask = np.concatenate([res.results[c]["mask"] for c in range(NCORES)], axis=0)
    idx = idx.reshape(B, Q, K_EFF).astype(np.int32, copy=False)
    mask = mask.reshape(B, Q, K_EFF).astype(bool)
    return idx, mask


if __name__ == "__main__":
    s = np.load("/tmp/scores.npy")
    i, m = kernel(s, 2048, 8192)
    print(i.shape, i.dtype, m.shape, m.dtype)


# revision 10
# speedup vs baseline: 1.1924x; 1.1038x over previous
"""AdaptiveTopKSelector Trainium2 kernel (8 NeuronCores, SPMD data-parallel).

Computes, for scores [4, 2048, 8192]:
  indices [4, 2048, 4096] int32 : per-row top-4096 indices, descending score,
                                  ties broken by lower index (jax.lax.top_k)
  mask    [4, 2048, 4096] bool  : prefix mask pos < k_adaptive(row)

Strategy: shard the 8192 (batch*seq_q) rows across 8 cores (1024 rows each,
8 tiles of [128 rows x 8192]).  Per tile, a normalized descending bitonic
sort (91 stages) runs on the Vector engine over (value fp32, index uint16)
pairs, with the two unconditional index base-copies offloaded to the Scalar
engine.  Equal-value runs are then re-ordered by index with 4 odd-even
fixup passes.  The adaptive k needs a global mean of per-row variances, so
phase A reduces per-row variance per shard and a single-scalar AllReduce
combines shard sums across the 8 cores.
"""

import os
import sys

for _p in ("/opt/trn_rl_repo", "/root/.axon_site/_ro/trn_rl_repo"):
    if os.path.isdir(_p) and _p not in sys.path:
        sys.path.append(_p)

import numpy as np

N = 8192          # row length (seq_kv)
K_EFF = 4096      # output k
ROWS = 8192       # total rows (4 * 2048)
NCORES = 8
ROWS_PER_CORE = ROWS // NCORES          # 1024
TILES = ROWS_PER_CORE // 128            # 8
FIXUP_PASSES = 4

_CACHE = {}


def _build():
    import concourse.bacc as bacc
    import concourse.mybir as mybir
    from concourse.tile import TileContext
    from bass_rust import ReduceOp as _ReduceOp

    f32 = mybir.dt.float32
    u16 = mybir.dt.uint16
    u8 = mybir.dt.uint8
    i32 = mybir.dt.int32
    Alu = mybir.AluOpType

    nc = bacc.Bacc("TRN2", target_bir_lowering=False, debug=False,
                   num_devices=NCORES)

    scores = nc.dram_tensor("scores", [ROWS_PER_CORE, N], f32,
                            kind="ExternalInput")
    idx_out = nc.dram_tensor("idx", [ROWS_PER_CORE, K_EFF], i32,
                             kind="ExternalOutput")
    mask_out = nc.dram_tensor("mask", [ROWS_PER_CORE, K_EFF], u8,
                              kind="ExternalOutput")
    vs_in = nc.dram_tensor("vs_in", [1, 1], f32)
    vs_out = nc.dram_tensor("vs_out", [1, 1], f32, addr_space="Shared")

    sc = scores.ap().rearrange("(t p) n -> t p n", p=128)
    io = idx_out.ap().rearrange("(t p) n -> t p n", p=128)
    mo = mask_out.ap().rearrange("(t p) n -> t p n", p=128)

    with TileContext(nc) as tc:
        with tc.tile_pool(name="big", bufs=1) as big:
            val0 = big.tile([128, N], f32, tag="val0")
            val1 = big.tile([128, N], f32, tag="val1")
            idx0 = big.tile([128, N], u16, tag="idx0")
            idx1 = big.tile([128, N], u16, tag="idx1")
            iota16 = big.tile([128, N], u16, tag="iota16")
            pos32 = big.tile([128, K_EFF], f32, tag="pos32")
            cbuf = big.tile([128, N // 2], u8, tag="cbuf")
            c2buf = big.tile([128, N // 2], u8, tag="c2buf")
            ftmp = big.tile([128, N // 2], u16, tag="ftmp")
            iostage = big.tile([128, K_EFF], i32, tag="iostage")
            mbuf = big.tile([128, K_EFF], u8, tag="mbuf")
            stats = big.tile([128, 32], f32, tag="stats")
            stats16 = big.tile([128, 4], u16, tag="stats16")
            # stats columns: 0:8 sumsq, 8:16 mean, 16:24 km1, 24.. scratch

            sumsq = stats[:, 0:8]
            means = stats[:, 8:16]
            km1 = stats[:, 16:24]
            scr = stats[:, 24:32]

            nc.gpsimd.iota(iota16[:], pattern=[[1, N]], base=0,
                           channel_multiplier=0,
                           allow_small_or_imprecise_dtypes=True)
            nc.gpsimd.iota(pos32[:], pattern=[[1, K_EFF]], base=0,
                           channel_multiplier=0,
                           allow_small_or_imprecise_dtypes=True)

            # ---------- phase A: per-row variance ----------
            for t in range(TILES):
                buf = val0 if t % 2 == 0 else val1
                tmp = val1 if t % 2 == 0 else val0
                nc.sync.dma_start(out=buf[:], in_=sc[t])
                nc.vector.reduce_sum(scr[:, 0:1], buf[:],
                                     axis=mybir.AxisListType.X)
                nc.vector.tensor_scalar(means[:, t:t + 1], scr[:, 0:1],
                                        1.0 / N, None, op0=Alu.mult)
                nc.vector.tensor_scalar(tmp[:], buf[:], means[:, t:t + 1],
                                        None, op0=Alu.subtract)
                nc.vector.tensor_mul(tmp[:], tmp[:], tmp[:])
                nc.vector.reduce_sum(sumsq[:, t:t + 1], tmp[:],
                                     axis=mybir.AxisListType.X)

            # var = sumsq/(N-1); shard var sum -> AllReduce -> global mean
            nc.vector.tensor_scalar(sumsq[:], sumsq[:], 1.0 / (N - 1), None,
                                    op0=Alu.mult)
            nc.vector.reduce_sum(scr[:, 1:2], sumsq[:],
                                 axis=mybir.AxisListType.X)
            nc.gpsimd.partition_all_reduce(scr[:, 2:3], scr[:, 1:2],
                                           channels=128,
                                           reduce_op=_ReduceOp.add)
            nc.sync.dma_start(out=vs_in[:], in_=scr[0:1, 2:3])
            nc.gpsimd.collective_compute(
                "AllReduce", Alu.add,
                replica_groups=[list(range(NCORES))],
                ins=[vs_in.ap()], outs=[vs_out.ap()])
            nc.sync.dma_start(out=scr[0:1, 3:4], in_=vs_out[:])
            nc.gpsimd.partition_broadcast(scr[:, 4:5], scr[0:1, 3:4])
            # inv = 1/(gsum/ROWS + 1e-8)
            nc.vector.tensor_scalar(scr[:, 5:6], scr[:, 4:5], 1.0 / ROWS,
                                    1e-8, op0=Alu.mult, op1=Alu.add)
            nc.vector.reciprocal(scr[:, 6:7], scr[:, 5:6])
            # km1 = clip(2048*(0.5 + 1/(1 + var*inv)), 256, 4096) - 1
            nc.vector.tensor_scalar(km1[:], sumsq[:], scr[:, 6:7], 1.0,
                                    op0=Alu.mult, op1=Alu.add)
            nc.vector.reciprocal(km1[:], km1[:])
            nc.vector.tensor_scalar(km1[:], km1[:], 0.5, 2048.0,
                                    op0=Alu.add, op1=Alu.mult)
            nc.vector.tensor_scalar(km1[:], km1[:], 4096.0, 256.0,
                                    op0=Alu.min, op1=Alu.max)
            nc.vector.tensor_scalar(km1[:], km1[:], 1.0, None,
                                    op0=Alu.subtract)

            # ---------- phase B: sort + outputs ----------
            def stage(va, ia, vb, ib, nva, nia, nvb, nib):
                # compare+exchange: max -> A position, min -> B position.
                # c = (va >= vb); value min/max on DVE; index base copies on
                # ACT (crossed), then predicated copies (straight) on DVE.
                free = list(va.shape[1:])
                L = 1
                for f in free:
                    L *= f
                cv = cbuf[:, 0:L]
                if len(free) == 1:
                    cap = cv
                elif len(free) == 2:
                    cap = cv.rearrange("p (a b) -> p a b", b=free[1])
                else:
                    cap = cv.rearrange("p (a b c) -> p a b c",
                                       b=free[1], c=free[2])
                nc.vector.tensor_tensor(cap, va, vb, op=Alu.is_ge)
                nc.vector.tensor_tensor(nvb, va, vb, op=Alu.min)
                nc.vector.tensor_tensor(nva, va, vb, op=Alu.max)
                nc.scalar.copy(nia, ib)
                nc.scalar.copy(nib, ia)
                nc.vector.copy_predicated(nia, cap, ia)
                nc.vector.copy_predicated(nib, cap, ib)

            for t in range(TILES):
                vbufs = [val0, val1]
                ibufs = [idx0, idx1]
                nc.sync.dma_start(out=val0[:], in_=sc[t])
                nc.vector.tensor_copy(idx0[:], iota16[:])
                s = 0  # current buffer index
                # ----- levels 0..11: Batcher odd-even merge ---------------
                for lev in range(12):
                    K = 1 << lev
                    vc, vn = vbufs[s], vbufs[1 - s]
                    ic, inx = ibufs[s], ibufs[1 - s]
                    g = vc[:].rearrange("p (b k) -> p b k", k=2 * K)
                    gi = ic[:].rearrange("p (b k) -> p b k", k=2 * K)
                    go = vn[:].rearrange("p (b k) -> p b k", k=2 * K)
                    goi = inx[:].rearrange("p (b k) -> p b k", k=2 * K)
                    stage(g[:, :, 0:K], gi[:, :, 0:K],
                          g[:, :, K:2 * K], gi[:, :, K:2 * K],
                          go[:, :, 0:K], goi[:, :, 0:K],
                          go[:, :, K:2 * K], goi[:, :, K:2 * K])
                    s = 1 - s
                    d = K // 2
                    while d >= 1:
                        vc, vn = vbufs[s], vbufs[1 - s]
                        ic, inx = ibufs[s], ibufs[1 - s]
                        g = vc[:].rearrange("p (b k) -> p b k", k=2 * K)
                        gi = ic[:].rearrange("p (b k) -> p b k", k=2 * K)
                        go = vn[:].rearrange("p (b k) -> p b k", k=2 * K)
                        goi = inx[:].rearrange("p (b k) -> p b k", k=2 * K)
                        # pairs (i, i+d) for i in chains within [d, 2K-d)
                        mid = g[:, :, d:2 * K - d].rearrange(
                            "p b (c j) -> p b c j", j=2 * d)
                        midi = gi[:, :, d:2 * K - d].rearrange(
                            "p b (c j) -> p b c j", j=2 * d)
                        mog = go[:, :, d:2 * K - d].rearrange(
                            "p b (c j) -> p b c j", j=2 * d)
                        moi = goi[:, :, d:2 * K - d].rearrange(
                            "p b (c j) -> p b c j", j=2 * d)
                        stage(mid[:, :, :, 0:d], midi[:, :, :, 0:d],
                              mid[:, :, :, d:2 * d], midi[:, :, :, d:2 * d],
                              mog[:, :, :, 0:d], moi[:, :, :, 0:d],
                              mog[:, :, :, d:2 * d], moi[:, :, :, d:2 * d])
                        # uncovered block ends [0,d) and [2K-d,2K): plain
                        # copies on the Scalar engine (it has headroom)
                        nc.scalar.copy(go[:, :, 0:d], g[:, :, 0:d])
                        nc.scalar.copy(goi[:, :, 0:d], gi[:, :, 0:d])
                        nc.scalar.copy(go[:, :, 2 * K - d:2 * K],
                                       g[:, :, 2 * K - d:2 * K])
                        nc.scalar.copy(goi[:, :, 2 * K - d:2 * K],
                                       gi[:, :, 2 * K - d:2 * K])
                        s = 1 - s
                        d //= 2
                # ----- level 12 (final): mirror + top-half substages ------
                K = N // 2
                vc, vn = vbufs[s], vbufs[1 - s]
                ic, inx = ibufs[s], ibufs[1 - s]
                g = vc[:].rearrange("p (b k) -> p b k", k=2 * K)
                gi = ic[:].rearrange("p (b k) -> p b k", k=2 * K)
                go = vn[:].rearrange("p (b k) -> p b k", k=2 * K)
                goi = inx[:].rearrange("p (b k) -> p b k", k=2 * K)

                def rev_half(x):
                    # reversed second half of the 2K block: 2K-1 .. K
                    return x[:, :, 2 * K - 1:K - 1:-1]
                stage(g[:, :, 0:K], gi[:, :, 0:K], rev_half(g), rev_half(gi),
                      go[:, :, 0:K], goi[:, :, 0:K],
                      rev_half(go), rev_half(goi))
                s = 1 - s
                j = K // 2
                while j >= 1:
                    vc, vn = vbufs[s], vbufs[1 - s]
                    ic, inx = ibufs[s], ibufs[1 - s]
                    g = vc[:, 0:K_EFF].rearrange("p (b k) -> p b k", k=2 * j)
                    gi = ic[:, 0:K_EFF].rearrange("p (b k) -> p b k", k=2 * j)
                    go = vn[:, 0:K_EFF].rearrange("p (b k) -> p b k", k=2 * j)
                    goi = inx[:, 0:K_EFF].rearrange("p (b k) -> p b k",
                                                    k=2 * j)
                    stage(g[:, :, 0:j], gi[:, :, 0:j],
                          g[:, :, j:2 * j], gi[:, :, j:2 * j],
                          go[:, :, 0:j], goi[:, :, 0:j],
                          go[:, :, j:2 * j], goi[:, :, j:2 * j])
                    s = 1 - s
                    j //= 2

                vf, iff = vbufs[s], ibufs[s]
                # ---------- boundary surgery ------------------------------
                # The bottom half was left unsorted (post-mirror bitonic).
                # If its max equals the value at position K_EFF-1 (a tie run
                # straddling the cut), the reference keeps the lowest-indexed
                # members: swap in the bottom's lowest-index tied element if
                # it beats the boundary element's index.  (The subsequent
                # fixup passes then order the run by index.)
                bs = stats[:, 28:32]  # scratch (phase-A scr cols are dead now)
                nc.vector.reduce_sum(bs[:, 0:1], vf[:, K_EFF:N],
                                     axis=mybir.AxisListType.X, op=Alu.max)
                nc.vector.tensor_scalar(cbuf[:, 0:N // 2], vf[:, K_EFF:N],
                                        bs[:, 0:1], None, op0=Alu.is_equal)
                nc.vector.memset(ftmp[:, 0:N // 2], 16383)
                nc.vector.copy_predicated(ftmp[:, 0:N // 2],
                                          cbuf[:, 0:N // 2], iff[:, K_EFF:N])
                nc.vector.reduce_sum(stats16[:, 0:1], ftmp[:, 0:N // 2],
                                     axis=mybir.AxisListType.X, op=Alu.min)
                # cond = (bottom_max == v[K_EFF-1]) & (rmin < idx[K_EFF-1])
                nc.vector.tensor_tensor(cbuf[:, 0:1], bs[:, 0:1],
                                        vf[:, K_EFF - 1:K_EFF], op=Alu.is_equal)
                nc.vector.tensor_tensor(c2buf[:, 0:1], stats16[:, 0:1],
                                        iff[:, K_EFF - 1:K_EFF], op=Alu.is_lt)
                nc.vector.tensor_tensor(cbuf[:, 0:1], cbuf[:, 0:1],
                                        c2buf[:, 0:1], op=Alu.mult)
                nc.vector.copy_predicated(iff[:, K_EFF - 1:K_EFF],
                                          cbuf[:, 0:1], stats16[:, 0:1])
                # ---------- tie fixup: order equal-value runs by index ----
                for p in range(FIXUP_PASSES):
                    par = p % 2
                    L = (K_EFF - par) // 2 * 2
                    npair = L // 2
                    va = vf[:, par:par + L].rearrange("p (a b) -> p a b", b=2)
                    ia = iff[:, par:par + L].rearrange("p (a b) -> p a b", b=2)
                    A_v, B_v = va[:, :, 0:1], va[:, :, 1:2]
                    A_i, B_i = ia[:, :, 0:1], ia[:, :, 1:2]
                    ceq = cbuf[:, 0:npair].rearrange("p (a b) -> p a b", b=1)
                    cgt = c2buf[:, 0:npair].rearrange("p (a b) -> p a b", b=1)
                    ft = ftmp[:, 0:npair].rearrange("p (a b) -> p a b", b=1)
                    nc.vector.tensor_tensor(ceq, A_v, B_v, op=Alu.is_equal)
                    nc.vector.tensor_tensor(cgt, A_i, B_i, op=Alu.is_gt)
                    nc.vector.tensor_tensor(ceq, ceq, cgt, op=Alu.mult)
                    nc.vector.tensor_copy(ft, A_i)
                    nc.vector.copy_predicated(A_i, ceq, B_i)
                    nc.vector.copy_predicated(B_i, ceq, ft)

                # ---------- outputs ----------
                nc.vector.tensor_copy(iostage[:], iff[:, 0:K_EFF])
                nc.sync.dma_start(out=io[t], in_=iostage[:])
                nc.vector.tensor_scalar(mbuf[:], pos32[:], km1[:, t:t + 1],
                                        None, op0=Alu.is_le)
                nc.sync.dma_start(out=mo[t], in_=mbuf[:])

    nc.compile()
    return nc


def _get_nc():
    if "nc" not in _CACHE:
        _CACHE["nc"] = _build()
    return _CACHE["nc"]


def kernel(scores, seq_q=None, seq_kv=None, **_ignored):
    from concourse.bass_utils import run_bass_kernel_spmd

    scores = np.ascontiguousarray(np.asarray(scores), dtype=np.float32)
    B, Q, Nk = scores.shape
    assert (B * Q, Nk) == (ROWS, N), f"unexpected shape {scores.shape}"
    rows = scores.reshape(ROWS, N)

    nc = _get_nc()
    in_maps = [
        {"scores": rows[c * ROWS_PER_CORE:(c + 1) * ROWS_PER_CORE]}
        for c in range(NCORES)
    ]
    res = run_bass_kernel_spmd(nc, in_maps, core_ids=list(range(NCORES)))
    idx = np.concatenate([res.results[c]["idx"] for c in range(NCORES)], axis=0)
    mask = np.concatenate([res.results[c]["mask"] for c in range(NCORES)], axis=0)
    idx = idx.reshape(B, Q, K_EFF).astype(np.int32, copy=False)
    mask = mask.reshape(B, Q, K_EFF).astype(bool)
    return idx, mask


if __name__ == "__main__":
    s = np.load("/tmp/scores.npy")
    i, m = kernel(s, 2048, 8192)
    print(i.shape, i.dtype, m.shape, m.dtype)


# revision 11
# speedup vs baseline: 1.1982x; 1.0049x over previous
"""AdaptiveTopKSelector Trainium2 kernel (8 NeuronCores, SPMD data-parallel).

Computes, for scores [4, 2048, 8192]:
  indices [4, 2048, 4096] int32 : per-row top-4096 indices, descending score,
                                  ties broken by lower index (jax.lax.top_k)
  mask    [4, 2048, 4096] bool  : prefix mask pos < k_adaptive(row)

Strategy: shard the 8192 (batch*seq_q) rows across 8 cores (1024 rows each,
8 tiles of [128 rows x 8192]).  Per tile, a normalized descending bitonic
sort (91 stages) runs on the Vector engine over (value fp32, index uint16)
pairs, with the two unconditional index base-copies offloaded to the Scalar
engine.  Equal-value runs are then re-ordered by index with 4 odd-even
fixup passes.  The adaptive k needs a global mean of per-row variances, so
phase A reduces per-row variance per shard and a single-scalar AllReduce
combines shard sums across the 8 cores.
"""

import os
import sys

for _p in ("/opt/trn_rl_repo", "/root/.axon_site/_ro/trn_rl_repo"):
    if os.path.isdir(_p) and _p not in sys.path:
        sys.path.append(_p)

import numpy as np

N = 8192          # row length (seq_kv)
K_EFF = 4096      # output k
ROWS = 8192       # total rows (4 * 2048)
NCORES = 8
ROWS_PER_CORE = ROWS // NCORES          # 1024
TILES = ROWS_PER_CORE // 128            # 8
FIXUP_PASSES = 4

_CACHE = {}


def _build():
    import concourse.bacc as bacc
    import concourse.mybir as mybir
    from concourse.tile import TileContext
    from bass_rust import ReduceOp as _ReduceOp

    f32 = mybir.dt.float32
    u16 = mybir.dt.uint16
    u8 = mybir.dt.uint8
    i32 = mybir.dt.int32
    Alu = mybir.AluOpType

    nc = bacc.Bacc("TRN2", target_bir_lowering=False, debug=False,
                   num_devices=NCORES)

    scores = nc.dram_tensor("scores", [ROWS_PER_CORE, N], f32,
                            kind="ExternalInput")
    idx_out = nc.dram_tensor("idx", [ROWS_PER_CORE, K_EFF], i32,
                             kind="ExternalOutput")
    mask_out = nc.dram_tensor("mask", [ROWS_PER_CORE, K_EFF], u8,
                              kind="ExternalOutput")
    vs_in = nc.dram_tensor("vs_in", [1, 1], f32)
    vs_out = nc.dram_tensor("vs_out", [1, 1], f32, addr_space="Shared")

    sc = scores.ap().rearrange("(t p) n -> t p n", p=128)
    io = idx_out.ap().rearrange("(t p) n -> t p n", p=128)
    mo = mask_out.ap().rearrange("(t p) n -> t p n", p=128)

    with TileContext(nc) as tc:
        with tc.tile_pool(name="big", bufs=1) as big:
            val0 = big.tile([128, N], f32, tag="val0")
            val1 = big.tile([128, N], f32, tag="val1")
            idx0 = big.tile([128, N], u16, tag="idx0")
            idx1 = big.tile([128, N], u16, tag="idx1")
            iota16 = big.tile([128, N], u16, tag="iota16")
            pos32 = big.tile([128, K_EFF], f32, tag="pos32")
            cbuf = big.tile([128, N // 2], u8, tag="cbuf")
            c2buf = big.tile([128, N // 2], u8, tag="c2buf")
            ftmp = big.tile([128, N // 2], u16, tag="ftmp")
            iostage = big.tile([128, K_EFF], i32, tag="iostage")
            mbuf = big.tile([128, K_EFF], u8, tag="mbuf")
            stats = big.tile([128, 32], f32, tag="stats")
            stats16 = big.tile([128, 4], u16, tag="stats16")
            # stats columns: 0:8 sumsq, 8:16 mean, 16:24 km1, 24.. scratch

            sumsq = stats[:, 0:8]
            means = stats[:, 8:16]
            km1 = stats[:, 16:24]
            scr = stats[:, 24:32]

            nc.gpsimd.iota(iota16[:], pattern=[[1, N]], base=0,
                           channel_multiplier=0,
                           allow_small_or_imprecise_dtypes=True)
            nc.gpsimd.iota(pos32[:], pattern=[[1, K_EFF]], base=0,
                           channel_multiplier=0,
                           allow_small_or_imprecise_dtypes=True)

            # ---------- phase A: per-row variance ----------
            for t in range(TILES):
                buf = val0 if t % 2 == 0 else val1
                tmp = val1 if t % 2 == 0 else val0
                nc.sync.dma_start(out=buf[:], in_=sc[t])
                nc.vector.reduce_sum(scr[:, 0:1], buf[:],
                                     axis=mybir.AxisListType.X)
                nc.vector.tensor_scalar(means[:, t:t + 1], scr[:, 0:1],
                                        1.0 / N, None, op0=Alu.mult)
                nc.vector.tensor_scalar(tmp[:], buf[:], means[:, t:t + 1],
                                        None, op0=Alu.subtract)
                nc.vector.tensor_mul(tmp[:], tmp[:], tmp[:])
                nc.vector.reduce_sum(sumsq[:, t:t + 1], tmp[:],
                                     axis=mybir.AxisListType.X)

            # var = sumsq/(N-1); shard var sum -> AllReduce -> global mean
            nc.vector.tensor_scalar(sumsq[:], sumsq[:], 1.0 / (N - 1), None,
                                    op0=Alu.mult)
            nc.vector.reduce_sum(scr[:, 1:2], sumsq[:],
                                 axis=mybir.AxisListType.X)
            nc.gpsimd.partition_all_reduce(scr[:, 2:3], scr[:, 1:2],
                                           channels=128,
                                           reduce_op=_ReduceOp.add)
            nc.sync.dma_start(out=vs_in[:], in_=scr[0:1, 2:3])
            nc.gpsimd.collective_compute(
                "AllReduce", Alu.add,
                replica_groups=[list(range(NCORES))],
                ins=[vs_in.ap()], outs=[vs_out.ap()])
            nc.sync.dma_start(out=scr[0:1, 3:4], in_=vs_out[:])
            nc.gpsimd.partition_broadcast(scr[:, 4:5], scr[0:1, 3:4])
            # inv = 1/(gsum/ROWS + 1e-8)
            nc.vector.tensor_scalar(scr[:, 5:6], scr[:, 4:5], 1.0 / ROWS,
                                    1e-8, op0=Alu.mult, op1=Alu.add)
            nc.vector.reciprocal(scr[:, 6:7], scr[:, 5:6])
            # km1 = clip(2048*(0.5 + 1/(1 + var*inv)), 256, 4096) - 1
            nc.vector.tensor_scalar(km1[:], sumsq[:], scr[:, 6:7], 1.0,
                                    op0=Alu.mult, op1=Alu.add)
            nc.vector.reciprocal(km1[:], km1[:])
            nc.vector.tensor_scalar(km1[:], km1[:], 0.5, 2048.0,
                                    op0=Alu.add, op1=Alu.mult)
            nc.vector.tensor_scalar(km1[:], km1[:], 4096.0, 256.0,
                                    op0=Alu.min, op1=Alu.max)
            nc.vector.tensor_scalar(km1[:], km1[:], 1.0, None,
                                    op0=Alu.subtract)

            # ---------- phase B: sort + outputs ----------
            def stage(va, ia, vb, ib, nva, nia, nvb, nib):
                # compare+exchange: max -> A position, min -> B position.
                # c = (va >= vb); value min/max on DVE; index base copies on
                # ACT (crossed), then predicated copies (straight) on DVE.
                free = list(va.shape[1:])
                L = 1
                for f in free:
                    L *= f
                cv = cbuf[:, 0:L]
                if len(free) == 1:
                    cap = cv
                elif len(free) == 2:
                    cap = cv.rearrange("p (a b) -> p a b", b=free[1])
                else:
                    cap = cv.rearrange("p (a b c) -> p a b c",
                                       b=free[1], c=free[2])
                nc.vector.tensor_tensor(cap, va, vb, op=Alu.is_ge)
                nc.vector.tensor_tensor(nvb, va, vb, op=Alu.min)
                nc.vector.tensor_tensor(nva, va, vb, op=Alu.max)
                nc.scalar.copy(nia, ib)
                nc.scalar.copy(nib, ia)
                nc.vector.copy_predicated(nia, cap, ia)
                nc.vector.copy_predicated(nib, cap, ib)

            for t in range(TILES):
                vbufs = [val0, val1]
                ibufs = [idx0, idx1]
                nc.sync.dma_start(out=val0[:], in_=sc[t])
                nc.scalar.copy(idx0[:], iota16[:])
                s = 0  # current buffer index
                # ----- levels 0..11: Batcher odd-even merge ---------------
                for lev in range(12):
                    K = 1 << lev
                    vc, vn = vbufs[s], vbufs[1 - s]
                    ic, inx = ibufs[s], ibufs[1 - s]
                    g = vc[:].rearrange("p (b k) -> p b k", k=2 * K)
                    gi = ic[:].rearrange("p (b k) -> p b k", k=2 * K)
                    go = vn[:].rearrange("p (b k) -> p b k", k=2 * K)
                    goi = inx[:].rearrange("p (b k) -> p b k", k=2 * K)
                    stage(g[:, :, 0:K], gi[:, :, 0:K],
                          g[:, :, K:2 * K], gi[:, :, K:2 * K],
                          go[:, :, 0:K], goi[:, :, 0:K],
                          go[:, :, K:2 * K], goi[:, :, K:2 * K])
                    s = 1 - s
                    d = K // 2
                    while d >= 1:
                        vc, vn = vbufs[s], vbufs[1 - s]
                        ic, inx = ibufs[s], ibufs[1 - s]
                        g = vc[:].rearrange("p (b k) -> p b k", k=2 * K)
                        gi = ic[:].rearrange("p (b k) -> p b k", k=2 * K)
                        go = vn[:].rearrange("p (b k) -> p b k", k=2 * K)
                        goi = inx[:].rearrange("p (b k) -> p b k", k=2 * K)
                        # pairs (i, i+d) for i in chains within [d, 2K-d)
                        mid = g[:, :, d:2 * K - d].rearrange(
                            "p b (c j) -> p b c j", j=2 * d)
                        midi = gi[:, :, d:2 * K - d].rearrange(
                            "p b (c j) -> p b c j", j=2 * d)
                        mog = go[:, :, d:2 * K - d].rearrange(
                            "p b (c j) -> p b c j", j=2 * d)
                        moi = goi[:, :, d:2 * K - d].rearrange(
                            "p b (c j) -> p b c j", j=2 * d)
                        stage(mid[:, :, :, 0:d], midi[:, :, :, 0:d],
                              mid[:, :, :, d:2 * d], midi[:, :, :, d:2 * d],
                              mog[:, :, :, 0:d], moi[:, :, :, 0:d],
                              mog[:, :, :, d:2 * d], moi[:, :, :, d:2 * d])
                        # uncovered block ends [0,d) and [2K-d,2K): plain
                        # copies on the Scalar engine (it has headroom)
                        nc.scalar.copy(go[:, :, 0:d], g[:, :, 0:d])
                        nc.scalar.copy(goi[:, :, 0:d], gi[:, :, 0:d])
                        nc.scalar.copy(go[:, :, 2 * K - d:2 * K],
                                       g[:, :, 2 * K - d:2 * K])
                        nc.scalar.copy(goi[:, :, 2 * K - d:2 * K],
                                       gi[:, :, 2 * K - d:2 * K])
                        s = 1 - s
                        d //= 2
                # ----- level 12 (final): mirror + top-half substages ------
                K = N // 2
                vc, vn = vbufs[s], vbufs[1 - s]
                ic, inx = ibufs[s], ibufs[1 - s]
                g = vc[:].rearrange("p (b k) -> p b k", k=2 * K)
                gi = ic[:].rearrange("p (b k) -> p b k", k=2 * K)
                go = vn[:].rearrange("p (b k) -> p b k", k=2 * K)
                goi = inx[:].rearrange("p (b k) -> p b k", k=2 * K)

                def rev_half(x):
                    # reversed second half of the 2K block: 2K-1 .. K
                    return x[:, :, 2 * K - 1:K - 1:-1]
                stage(g[:, :, 0:K], gi[:, :, 0:K], rev_half(g), rev_half(gi),
                      go[:, :, 0:K], goi[:, :, 0:K],
                      rev_half(go), rev_half(goi))
                s = 1 - s
                j = K // 2
                while j >= 1:
                    vc, vn = vbufs[s], vbufs[1 - s]
                    ic, inx = ibufs[s], ibufs[1 - s]
                    g = vc[:, 0:K_EFF].rearrange("p (b k) -> p b k", k=2 * j)
                    gi = ic[:, 0:K_EFF].rearrange("p (b k) -> p b k", k=2 * j)
                    go = vn[:, 0:K_EFF].rearrange("p (b k) -> p b k", k=2 * j)
                    goi = inx[:, 0:K_EFF].rearrange("p (b k) -> p b k",
                                                    k=2 * j)
                    stage(g[:, :, 0:j], gi[:, :, 0:j],
                          g[:, :, j:2 * j], gi[:, :, j:2 * j],
                          go[:, :, 0:j], goi[:, :, 0:j],
                          go[:, :, j:2 * j], goi[:, :, j:2 * j])
                    s = 1 - s
                    j //= 2

                vf, iff = vbufs[s], ibufs[s]
                # ---------- boundary surgery ------------------------------
                # The bottom half was left unsorted (post-mirror bitonic).
                # If its max equals the value at position K_EFF-1 (a tie run
                # straddling the cut), the reference keeps the lowest-indexed
                # members: swap in the bottom's lowest-index tied element if
                # it beats the boundary element's index.  (The subsequent
                # fixup passes then order the run by index.)
                bs = stats[:, 28:32]  # scratch (phase-A scr cols are dead now)
                nc.vector.reduce_sum(bs[:, 0:1], vf[:, K_EFF:N],
                                     axis=mybir.AxisListType.X, op=Alu.max)
                nc.vector.tensor_scalar(cbuf[:, 0:N // 2], vf[:, K_EFF:N],
                                        bs[:, 0:1], None, op0=Alu.is_equal)
                nc.vector.memset(ftmp[:, 0:N // 2], 16383)
                nc.vector.copy_predicated(ftmp[:, 0:N // 2],
                                          cbuf[:, 0:N // 2], iff[:, K_EFF:N])
                nc.vector.reduce_sum(stats16[:, 0:1], ftmp[:, 0:N // 2],
                                     axis=mybir.AxisListType.X, op=Alu.min)
                # cond = (bottom_max == v[K_EFF-1]) & (rmin < idx[K_EFF-1])
                nc.vector.tensor_tensor(cbuf[:, 0:1], bs[:, 0:1],
                                        vf[:, K_EFF - 1:K_EFF], op=Alu.is_equal)
                nc.vector.tensor_tensor(c2buf[:, 0:1], stats16[:, 0:1],
                                        iff[:, K_EFF - 1:K_EFF], op=Alu.is_lt)
                nc.vector.tensor_tensor(cbuf[:, 0:1], cbuf[:, 0:1],
                                        c2buf[:, 0:1], op=Alu.mult)
                nc.vector.copy_predicated(iff[:, K_EFF - 1:K_EFF],
                                          cbuf[:, 0:1], stats16[:, 0:1])
                # ---------- tie fixup: order equal-value runs by index ----
                for p in range(FIXUP_PASSES):
                    par = p % 2
                    L = (K_EFF - par) // 2 * 2
                    npair = L // 2
                    va = vf[:, par:par + L].rearrange("p (a b) -> p a b", b=2)
                    ia = iff[:, par:par + L].rearrange("p (a b) -> p a b", b=2)
                    A_v, B_v = va[:, :, 0:1], va[:, :, 1:2]
                    A_i, B_i = ia[:, :, 0:1], ia[:, :, 1:2]
                    ceq = cbuf[:, 0:npair].rearrange("p (a b) -> p a b", b=1)
                    cgt = c2buf[:, 0:npair].rearrange("p (a b) -> p a b", b=1)
                    ft = ftmp[:, 0:npair].rearrange("p (a b) -> p a b", b=1)
                    nc.vector.tensor_tensor(ceq, A_v, B_v, op=Alu.is_equal)
                    nc.vector.tensor_tensor(cgt, A_i, B_i, op=Alu.is_gt)
                    nc.vector.tensor_tensor(ceq, ceq, cgt, op=Alu.mult)
                    nc.scalar.copy(ft, A_i)
                    nc.vector.copy_predicated(A_i, ceq, B_i)
                    nc.vector.copy_predicated(B_i, ceq, ft)

                # ---------- outputs ----------
                nc.scalar.copy(iostage[:], iff[:, 0:K_EFF])
                nc.sync.dma_start(out=io[t], in_=iostage[:])
                nc.vector.tensor_scalar(mbuf[:], pos32[:], km1[:, t:t + 1],
                                        None, op0=Alu.is_le)
                nc.sync.dma_start(out=mo[t], in_=mbuf[:])

    nc.compile()
    return nc


def _get_nc():
    if "nc" not in _CACHE:
        _CACHE["nc"] = _build()
    return _CACHE["nc"]


def kernel(scores, seq_q=None, seq_kv=None, **_ignored):
    from concourse.bass_utils import run_bass_kernel_spmd

    scores = np.ascontiguousarray(np.asarray(scores), dtype=np.float32)
    B, Q, Nk = scores.shape
    assert (B * Q, Nk) == (ROWS, N), f"unexpected shape {scores.shape}"
    rows = scores.reshape(ROWS, N)

    nc = _get_nc()
    in_maps = [
        {"scores": rows[c * ROWS_PER_CORE:(c + 1) * ROWS_PER_CORE]}
        for c in range(NCORES)
    ]
    res = run_bass_kernel_spmd(nc, in_maps, core_ids=list(range(NCORES)))
    idx = np.concatenate([res.results[c]["idx"] for c in range(NCORES)], axis=0)
    mask = np.concatenate([res.results[c]["mask"] for c in range(NCORES)], axis=0)
    idx = idx.reshape(B, Q, K_EFF).astype(np.int32, copy=False)
    mask = mask.reshape(B, Q, K_EFF).astype(bool)
    return idx, mask


if __name__ == "__main__":
    s = np.load("/tmp/scores.npy")
    i, m = kernel(s, 2048, 8192)
    print(i.shape, i.dtype, m.shape, m.dtype)


# revision 12
# speedup vs baseline: 1.2072x; 1.0075x over previous
"""AdaptiveTopKSelector Trainium2 kernel (8 NeuronCores, SPMD data-parallel).

Computes, for scores [4, 2048, 8192]:
  indices [4, 2048, 4096] int32 : per-row top-4096 indices, descending score,
                                  ties broken by lower index (jax.lax.top_k)
  mask    [4, 2048, 4096] bool  : prefix mask pos < k_adaptive(row)

Strategy: shard the 8192 (batch*seq_q) rows across 8 cores (1024 rows each,
8 tiles of [128 rows x 8192]).  Per tile, a normalized descending bitonic
sort (91 stages) runs on the Vector engine over (value fp32, index uint16)
pairs, with the two unconditional index base-copies offloaded to the Scalar
engine.  Equal-value runs are then re-ordered by index with 4 odd-even
fixup passes.  The adaptive k needs a global mean of per-row variances, so
phase A reduces per-row variance per shard and a single-scalar AllReduce
combines shard sums across the 8 cores.
"""

import os
import sys

for _p in ("/opt/trn_rl_repo", "/root/.axon_site/_ro/trn_rl_repo"):
    if os.path.isdir(_p) and _p not in sys.path:
        sys.path.append(_p)

import numpy as np

N = 8192          # row length (seq_kv)
K_EFF = 4096      # output k
ROWS = 8192       # total rows (4 * 2048)
NCORES = 8
ROWS_PER_CORE = ROWS // NCORES          # 1024
TILES = ROWS_PER_CORE // 128            # 8
FIXUP_PASSES = 4

_CACHE = {}


def _build():
    import concourse.bacc as bacc
    import concourse.mybir as mybir
    from concourse.tile import TileContext
    from bass_rust import ReduceOp as _ReduceOp

    f32 = mybir.dt.float32
    u16 = mybir.dt.uint16
    u8 = mybir.dt.uint8
    i32 = mybir.dt.int32
    Alu = mybir.AluOpType

    nc = bacc.Bacc("TRN2", target_bir_lowering=False, debug=False,
                   num_devices=NCORES)

    scores = nc.dram_tensor("scores", [ROWS_PER_CORE, N], f32,
                            kind="ExternalInput")
    idx_out = nc.dram_tensor("idx", [ROWS_PER_CORE, K_EFF], i32,
                             kind="ExternalOutput")
    mask_out = nc.dram_tensor("mask", [ROWS_PER_CORE, K_EFF], u8,
                              kind="ExternalOutput")
    vs_in = nc.dram_tensor("vs_in", [1, 1], f32)
    vs_out = nc.dram_tensor("vs_out", [1, 1], f32, addr_space="Shared")

    sc = scores.ap().rearrange("(t p) n -> t p n", p=128)
    io = idx_out.ap().rearrange("(t p) n -> t p n", p=128)
    mo = mask_out.ap().rearrange("(t p) n -> t p n", p=128)

    with TileContext(nc) as tc:
        with tc.tile_pool(name="big", bufs=1) as big:
            val0 = big.tile([128, N], f32, tag="val0")
            val1 = big.tile([128, N], f32, tag="val1")
            idx0 = big.tile([128, N], u16, tag="idx0")
            idx1 = big.tile([128, N], u16, tag="idx1")
            iota16 = big.tile([128, N], u16, tag="iota16")
            pos32 = big.tile([128, K_EFF], f32, tag="pos32")
            cbuf = big.tile([128, N // 2], u8, tag="cbuf")
            c2buf = big.tile([128, N // 2], u8, tag="c2buf")
            ftmp = big.tile([128, N // 2], u16, tag="ftmp")
            iostage = big.tile([128, K_EFF], i32, tag="iostage")
            mbuf = big.tile([128, K_EFF], u8, tag="mbuf")
            stats = big.tile([128, 32], f32, tag="stats")
            stats16 = big.tile([128, 4], u16, tag="stats16")
            # stats columns: 0:8 sumsq, 8:16 mean, 16:24 km1, 24.. scratch

            sumsq = stats[:, 0:8]
            means = stats[:, 8:16]
            km1 = stats[:, 16:24]
            scr = stats[:, 24:32]

            nc.gpsimd.iota(iota16[:], pattern=[[1, N]], base=0,
                           channel_multiplier=0,
                           allow_small_or_imprecise_dtypes=True)
            nc.gpsimd.iota(pos32[:], pattern=[[1, K_EFF]], base=0,
                           channel_multiplier=0,
                           allow_small_or_imprecise_dtypes=True)

            # ---------- phase A: per-row variance ----------
            for t in range(TILES):
                buf = val0 if t % 2 == 0 else val1
                tmp = val1 if t % 2 == 0 else val0
                nc.sync.dma_start(out=buf[:], in_=sc[t])
                nc.vector.reduce_sum(scr[:, 0:1], buf[:],
                                     axis=mybir.AxisListType.X)
                nc.vector.tensor_scalar(means[:, t:t + 1], scr[:, 0:1],
                                        -1.0 / N, None, op0=Alu.mult)
                # (x - mean)^2 with the sum as accumulate, all on ScalarE
                nc.scalar.activation(tmp[:], buf[:],
                                     mybir.ActivationFunctionType.Square,
                                     bias=means[:, t:t + 1],
                                     accum_out=sumsq[:, t:t + 1])

            # var = sumsq/(N-1); shard var sum -> AllReduce -> global mean
            nc.vector.tensor_scalar(sumsq[:], sumsq[:], 1.0 / (N - 1), None,
                                    op0=Alu.mult)
            nc.vector.reduce_sum(scr[:, 1:2], sumsq[:],
                                 axis=mybir.AxisListType.X)
            nc.gpsimd.partition_all_reduce(scr[:, 2:3], scr[:, 1:2],
                                           channels=128,
                                           reduce_op=_ReduceOp.add)
            nc.sync.dma_start(out=vs_in[:], in_=scr[0:1, 2:3])
            nc.gpsimd.collective_compute(
                "AllReduce", Alu.add,
                replica_groups=[list(range(NCORES))],
                ins=[vs_in.ap()], outs=[vs_out.ap()])
            nc.sync.dma_start(out=scr[0:1, 3:4], in_=vs_out[:])
            nc.gpsimd.partition_broadcast(scr[:, 4:5], scr[0:1, 3:4])
            # inv = 1/(gsum/ROWS + 1e-8)
            nc.vector.tensor_scalar(scr[:, 5:6], scr[:, 4:5], 1.0 / ROWS,
                                    1e-8, op0=Alu.mult, op1=Alu.add)
            nc.vector.reciprocal(scr[:, 6:7], scr[:, 5:6])
            # km1 = clip(2048*(0.5 + 1/(1 + var*inv)), 256, 4096) - 1
            nc.vector.tensor_scalar(km1[:], sumsq[:], scr[:, 6:7], 1.0,
                                    op0=Alu.mult, op1=Alu.add)
            nc.vector.reciprocal(km1[:], km1[:])
            nc.vector.tensor_scalar(km1[:], km1[:], 0.5, 2048.0,
                                    op0=Alu.add, op1=Alu.mult)
            nc.vector.tensor_scalar(km1[:], km1[:], 4096.0, 256.0,
                                    op0=Alu.min, op1=Alu.max)
            nc.vector.tensor_scalar(km1[:], km1[:], 1.0, None,
                                    op0=Alu.subtract)

            # ---------- phase B: sort + outputs ----------
            def stage(va, ia, vb, ib, nva, nia, nvb, nib):
                # compare+exchange: max -> A position, min -> B position.
                # c = (va >= vb); value min/max on DVE; index base copies on
                # ACT (crossed), then predicated copies (straight) on DVE.
                free = list(va.shape[1:])
                L = 1
                for f in free:
                    L *= f
                cv = cbuf[:, 0:L]
                if len(free) == 1:
                    cap = cv
                elif len(free) == 2:
                    cap = cv.rearrange("p (a b) -> p a b", b=free[1])
                else:
                    cap = cv.rearrange("p (a b c) -> p a b c",
                                       b=free[1], c=free[2])
                nc.vector.tensor_tensor(cap, va, vb, op=Alu.is_ge)
                nc.vector.tensor_tensor(nvb, va, vb, op=Alu.min)
                nc.vector.tensor_tensor(nva, va, vb, op=Alu.max)
                nc.scalar.copy(nia, ib)
                nc.scalar.copy(nib, ia)
                nc.vector.copy_predicated(nia, cap, ia)
                nc.vector.copy_predicated(nib, cap, ib)

            for t in range(TILES):
                vbufs = [val0, val1]
                ibufs = [idx0, idx1]
                nc.sync.dma_start(out=val0[:], in_=sc[t])
                nc.scalar.copy(idx0[:], iota16[:])
                s = 0  # current buffer index
                # ----- levels 0..11: Batcher odd-even merge ---------------
                for lev in range(12):
                    K = 1 << lev
                    vc, vn = vbufs[s], vbufs[1 - s]
                    ic, inx = ibufs[s], ibufs[1 - s]
                    g = vc[:].rearrange("p (b k) -> p b k", k=2 * K)
                    gi = ic[:].rearrange("p (b k) -> p b k", k=2 * K)
                    go = vn[:].rearrange("p (b k) -> p b k", k=2 * K)
                    goi = inx[:].rearrange("p (b k) -> p b k", k=2 * K)
                    stage(g[:, :, 0:K], gi[:, :, 0:K],
                          g[:, :, K:2 * K], gi[:, :, K:2 * K],
                          go[:, :, 0:K], goi[:, :, 0:K],
                          go[:, :, K:2 * K], goi[:, :, K:2 * K])
                    s = 1 - s
                    d = K // 2
                    while d >= 1:
                        vc, vn = vbufs[s], vbufs[1 - s]
                        ic, inx = ibufs[s], ibufs[1 - s]
                        g = vc[:].rearrange("p (b k) -> p b k", k=2 * K)
                        gi = ic[:].rearrange("p (b k) -> p b k", k=2 * K)
                        go = vn[:].rearrange("p (b k) -> p b k", k=2 * K)
                        goi = inx[:].rearrange("p (b k) -> p b k", k=2 * K)
                        # pairs (i, i+d) for i in chains within [d, 2K-d)
                        mid = g[:, :, d:2 * K - d].rearrange(
                            "p b (c j) -> p b c j", j=2 * d)
                        midi = gi[:, :, d:2 * K - d].rearrange(
                            "p b (c j) -> p b c j", j=2 * d)
                        mog = go[:, :, d:2 * K - d].rearrange(
                            "p b (c j) -> p b c j", j=2 * d)
                        moi = goi[:, :, d:2 * K - d].rearrange(
                            "p b (c j) -> p b c j", j=2 * d)
                        stage(mid[:, :, :, 0:d], midi[:, :, :, 0:d],
                              mid[:, :, :, d:2 * d], midi[:, :, :, d:2 * d],
                              mog[:, :, :, 0:d], moi[:, :, :, 0:d],
                              mog[:, :, :, d:2 * d], moi[:, :, :, d:2 * d])
                        # uncovered block ends [0,d) and [2K-d,2K): plain
                        # copies on the Scalar engine (it has headroom)
                        nc.scalar.copy(go[:, :, 0:d], g[:, :, 0:d])
                        nc.scalar.copy(goi[:, :, 0:d], gi[:, :, 0:d])
                        nc.scalar.copy(go[:, :, 2 * K - d:2 * K],
                                       g[:, :, 2 * K - d:2 * K])
                        nc.scalar.copy(goi[:, :, 2 * K - d:2 * K],
                                       gi[:, :, 2 * K - d:2 * K])
                        s = 1 - s
                        d //= 2
                # ----- level 12 (final): mirror + top-half substages ------
                K = N // 2
                vc, vn = vbufs[s], vbufs[1 - s]
                ic, inx = ibufs[s], ibufs[1 - s]
                g = vc[:].rearrange("p (b k) -> p b k", k=2 * K)
                gi = ic[:].rearrange("p (b k) -> p b k", k=2 * K)
                go = vn[:].rearrange("p (b k) -> p b k", k=2 * K)
                goi = inx[:].rearrange("p (b k) -> p b k", k=2 * K)

                def rev_half(x):
                    # reversed second half of the 2K block: 2K-1 .. K
                    return x[:, :, 2 * K - 1:K - 1:-1]
                stage(g[:, :, 0:K], gi[:, :, 0:K], rev_half(g), rev_half(gi),
                      go[:, :, 0:K], goi[:, :, 0:K],
                      rev_half(go), rev_half(goi))
                s = 1 - s
                j = K // 2
                while j >= 1:
                    vc, vn = vbufs[s], vbufs[1 - s]
                    ic, inx = ibufs[s], ibufs[1 - s]
                    g = vc[:, 0:K_EFF].rearrange("p (b k) -> p b k", k=2 * j)
                    gi = ic[:, 0:K_EFF].rearrange("p (b k) -> p b k", k=2 * j)
                    go = vn[:, 0:K_EFF].rearrange("p (b k) -> p b k", k=2 * j)
                    goi = inx[:, 0:K_EFF].rearrange("p (b k) -> p b k",
                                                    k=2 * j)
                    stage(g[:, :, 0:j], gi[:, :, 0:j],
                          g[:, :, j:2 * j], gi[:, :, j:2 * j],
                          go[:, :, 0:j], goi[:, :, 0:j],
                          go[:, :, j:2 * j], goi[:, :, j:2 * j])
                    s = 1 - s
                    j //= 2

                vf, iff = vbufs[s], ibufs[s]
                # ---------- boundary surgery ------------------------------
                # The bottom half was left unsorted (post-mirror bitonic).
                # If its max equals the value at position K_EFF-1 (a tie run
                # straddling the cut), the reference keeps the lowest-indexed
                # members: swap in the bottom's lowest-index tied element if
                # it beats the boundary element's index.  (The subsequent
                # fixup passes then order the run by index.)
                bs = stats[:, 28:32]  # scratch (phase-A scr cols are dead now)
                nc.vector.reduce_sum(bs[:, 0:1], vf[:, K_EFF:N],
                                     axis=mybir.AxisListType.X, op=Alu.max)
                nc.vector.tensor_scalar(cbuf[:, 0:N // 2], vf[:, K_EFF:N],
                                        bs[:, 0:1], None, op0=Alu.is_equal)
                nc.vector.memset(ftmp[:, 0:N // 2], 16383)
                nc.vector.copy_predicated(ftmp[:, 0:N // 2],
                                          cbuf[:, 0:N // 2], iff[:, K_EFF:N])
                nc.vector.reduce_sum(stats16[:, 0:1], ftmp[:, 0:N // 2],
                                     axis=mybir.AxisListType.X, op=Alu.min)
                # cond = (bottom_max == v[K_EFF-1]) & (rmin < idx[K_EFF-1])
                nc.vector.tensor_tensor(cbuf[:, 0:1], bs[:, 0:1],
                                        vf[:, K_EFF - 1:K_EFF], op=Alu.is_equal)
                nc.vector.tensor_tensor(c2buf[:, 0:1], stats16[:, 0:1],
                                        iff[:, K_EFF - 1:K_EFF], op=Alu.is_lt)
                nc.vector.tensor_tensor(cbuf[:, 0:1], cbuf[:, 0:1],
                                        c2buf[:, 0:1], op=Alu.mult)
                nc.vector.copy_predicated(iff[:, K_EFF - 1:K_EFF],
                                          cbuf[:, 0:1], stats16[:, 0:1])
                # ---------- tie fixup: order equal-value runs by index ----
                for p in range(FIXUP_PASSES):
                    par = p % 2
                    L = (K_EFF - par) // 2 * 2
                    npair = L // 2
                    va = vf[:, par:par + L].rearrange("p (a b) -> p a b", b=2)
                    ia = iff[:, par:par + L].rearrange("p (a b) -> p a b", b=2)
                    A_v, B_v = va[:, :, 0:1], va[:, :, 1:2]
                    A_i, B_i = ia[:, :, 0:1], ia[:, :, 1:2]
                    ceq = cbuf[:, 0:npair].rearrange("p (a b) -> p a b", b=1)
                    cgt = c2buf[:, 0:npair].rearrange("p (a b) -> p a b", b=1)
                    ft = ftmp[:, 0:npair].rearrange("p (a b) -> p a b", b=1)
                    nc.vector.tensor_tensor(ceq, A_v, B_v, op=Alu.is_equal)
                    nc.vector.tensor_tensor(cgt, A_i, B_i, op=Alu.is_gt)
                    nc.vector.tensor_tensor(ceq, ceq, cgt, op=Alu.mult)
                    nc.scalar.copy(ft, A_i)
                    nc.vector.copy_predicated(A_i, ceq, B_i)
                    nc.vector.copy_predicated(B_i, ceq, ft)

                # ---------- outputs ----------
                nc.scalar.copy(iostage[:], iff[:, 0:K_EFF])
                nc.sync.dma_start(out=io[t], in_=iostage[:])
                nc.vector.tensor_scalar(mbuf[:], pos32[:], km1[:, t:t + 1],
                                        None, op0=Alu.is_le)
                nc.sync.dma_start(out=mo[t], in_=mbuf[:])

    nc.compile()
    return nc


def _get_nc():
    if "nc" not in _CACHE:
        _CACHE["nc"] = _build()
    return _CACHE["nc"]


def kernel(scores, seq_q=None, seq_kv=None, **_ignored):
    from concourse.bass_utils import run_bass_kernel_spmd

    scores = np.ascontiguousarray(np.asarray(scores), dtype=np.float32)
    B, Q, Nk = scores.shape
    assert (B * Q, Nk) == (ROWS, N), f"unexpected shape {scores.shape}"
    rows = scores.reshape(ROWS, N)

    nc = _get_nc()
    in_maps = [
        {"scores": rows[c * ROWS_PER_CORE:(c + 1) * ROWS_PER_CORE]}
        for c in range(NCORES)
    ]
    res = run_bass_kernel_spmd(nc, in_maps, core_ids=list(range(NCORES)))
    idx = np.concatenate([res.results[c]["idx"] for c in range(NCORES)], axis=0)
    mask = np.concatenate([res.results[c]["mask"] for c in range(NCORES)], axis=0)
    idx = idx.reshape(B, Q, K_EFF).astype(np.int32, copy=False)
    mask = mask.reshape(B, Q, K_EFF).astype(bool)
    return idx, mask


if __name__ == "__main__":
    s = np.load("/tmp/scores.npy")
    i, m = kernel(s, 2048, 8192)
    print(i.shape, i.dtype, m.shape, m.dtype)
